# revision 1
# baseline (speedup 1.0000x reference)
"""Trainium2 Bass kernel for a 3-layer spiking net (snntorch-style Leaky/LIF).

Math (per timestep t, eval mode):
    cur1 = x_t @ w1.T + b1
    mem1 = 0.9*mem1 + cur1 - (mem1_prev > 1)        # reset-by-subtract
    spk1 = (mem1 > 1)
    cur2 = spk1 @ w2.T + b2
    mem2 = 0.85*mem2 + cur2 - (mem2_prev > 1)
    spk2 = (mem2 > 1)
    out_t = spk2 @ w3.T + b3

Strategy:
  - Data-parallel over batch: B=64 -> 8 cores x 8 (sharding hint).
  - The three matmuls do not depend on the recurrence, so they are batched
    over all T in chunks; only the elementwise LIF updates are sequential.
  - The mem trajectories sit on the fp32 grid within ~1 ulp of the firing
    threshold, so cur1/cur2 need sub-half-ulp accuracy vs the fp32
    reference; anything coarser flips spikes and cascades (bf16 hi/lo x
    splits measured 5e-3..5e-1 output rel-err).
  - Matmul 1 runs as three fp16 passes at the bf16 PE rate (4/3x faster
    than fp32): xh@w1h + xl@w1h + (xh>>10)@(w1l<<10).  The opposite 2^10
    exponent shifts keep w1's lo part in fp16 normal range (unshifted it
    is all denormal, truncating to a 2^-9-ulp grid) while products land
    at natural scale, so all 12 matmuls of a j-tile share one PSUM
    accumulation group.  Measured band ~2.7e-7: output rel-err identical
    to the fp32-MM1 build.
  - Matmul 2 runs as two bf16 passes (w2 split hi+lo): spikes are exactly
    representable in bf16, so the only inexactness is fp32 accumulation
    order -- measured 2e-9 output rel-err, at 2x the fp32 PE rate.
  - Matmul 3 feeds the output directly (no threshold), so a single bf16
    pass suffices (measured 1.5e-3 rel-err, gate is 2e-2).
  - All operand transposes / fp16-bf16 splits are done host-side in
    kernel(); the device sees ready-to-stream layouts.
  - Scan layout: [128 partitions = h%128, free = t*64 + (h//128)*8 + b],
    so each timestep is one fat [128, 64] tile.  The LIF update is two
    stt ops per layer-step on the negated membrane (see emit_scan1),
    written in place over the cur tiles; spikes are extracted in batched
    is_lt ops off the serial chain.
  - Chunks follow a variable schedule (small head/tail chunks) and the
    emission is software-pipelined: scan1(c+1) runs ahead of scan2(c) on
    the DVE, and all matmul-3 tiles run in the epilogue -- they are the
    only PE work free of serial-scan dependencies, so they fill the
    pipeline-drain window while layer-2's last chains finish.
"""

import sys

for _p in ("/opt/trn_rl_repo", "/root/.axon_site/_ro/pypackages"):
    if _p not in sys.path:
        sys.path.insert(0, _p)

import ml_dtypes
import numpy as np

import concourse.bass as bass
import concourse.mybir as mybir
from concourse import bacc, tile
from concourse.bass_utils import run_bass_kernel_spmd

F32 = mybir.dt.float32
F16 = mybir.dt.float16
BF16 = mybir.dt.bfloat16
ALU = mybir.AluOpType
ACTF = mybir.ActivationFunctionType

# Problem shape (hardcoded; harness runs kernel.py standalone).
T, B, I, H1, H2, O = 256, 64, 512, 1024, 1024, 256
NCORES = 8
BL = B // NCORES          # batch per core
BETA1, BETA2 = 0.9, 0.85
TC = 64                   # timesteps per pipeline chunk
KI = I // 128             # K-tiles for matmul 1 (4)
J1 = H1 // 128            # M-tiles for layer 1 (8)
J2 = H2 // 128            # M-tiles for layer 2 (8)


def build(n_t=T, sched=None, trace_sim=False, opts=None):
    """Build the per-core SPMD program. Identical on all cores.
    sched: list of per-chunk timestep counts (multiples of 16, sums to n_t).
    Small tail chunks keep the PE fed during the final serial scans."""
    if sched is None:
        sched = (opts or {}).get("sched") or [16, 48, 48, 48, 32, 32, 16, 16]
    assert sum(sched) == n_t and all(s % 8 == 0 for s in sched)
    nc = bacc.Bacc("TRN2", target_bir_lowering=False, debug=False)

    xh = nc.declare_dram_parameter("xh", [I, n_t * BL], F16, isOutput=False)
    xl = nc.declare_dram_parameter("xl", [I, n_t * BL], F16, isOutput=False)
    xh10 = nc.declare_dram_parameter("xh10", [I, n_t * BL], F16,
                                     isOutput=False)
    w1h = nc.declare_dram_parameter("w1h", [I, H1], F16, isOutput=False)
    w1l10 = nc.declare_dram_parameter("w1l10", [I, H1], F16, isOutput=False)
    w2h = nc.declare_dram_parameter("w2h", [H1, H2], BF16, isOutput=False)
    w2l = nc.declare_dram_parameter("w2l", [H1, H2], BF16, isOutput=False)
    w3h = nc.declare_dram_parameter("w3h", [H2, O], BF16, isOutput=False)
    y = nc.declare_dram_parameter("y", [n_t * BL, O], F32, isOutput=True)

    with tile.TileContext(nc, trace_sim=trace_sim) as tc_ctx:
        _body(nc, tc_ctx, (xh, xl, xh10), (w1h, w1l10), w2h, w2l, w3h, y,
              n_t, sched, opts or {})
    nc.compile()
    return nc


def _body(nc, tc_ctx, x_ds, w1_ds, w2h_d, w2l_d, w3h_d, y,
          n_t, sched, opts):
    nch = len(sched)
    ntb0 = sched[0] * BL
    import contextlib

    ctx = contextlib.ExitStack()
    with ctx:
        cb = opts.get("cur_bufs", 2)
        sb = opts.get("spk_bufs", 2)
        pb1, pb2, pb3 = opts.get("psum_bufs", (3, 3, 2))
        wsb = ctx.enter_context(tc_ctx.tile_pool(name="wsb", bufs=1))
        xt_pool = ctx.enter_context(tc_ctx.tile_pool(name="xt", bufs=opts.get("xt_bufs", 2)))
        cur1_pool = ctx.enter_context(tc_ctx.tile_pool(name="cur1", bufs=cb))
        spk1_pool = ctx.enter_context(tc_ctx.tile_pool(name="spk1", bufs=2))
        cur2_pool = ctx.enter_context(tc_ctx.tile_pool(name="cur2", bufs=cb))
        spk2_pool = ctx.enter_context(tc_ctx.tile_pool(name="spk2",
                                                       bufs=len(sched)))
        out_pool = ctx.enter_context(tc_ctx.tile_pool(name="outp", bufs=3))
        pp1 = ctx.enter_context(tc_ctx.tile_pool(name="pp1", bufs=pb1, space="PSUM"))
        pp2 = ctx.enter_context(tc_ctx.tile_pool(name="pp2", bufs=pb2, space="PSUM"))
        pp3 = ctx.enter_context(tc_ctx.tile_pool(name="pp3", bufs=pb3, space="PSUM"))

        # ---- weight loads (pre-transposed on host) ----------------------
        # Ordered so chunk-0's first matmuls can start as soon as the first
        # small pieces land: interleave x(0)-io / w1-io, then w2 (needed
        # at ~P2(0)), then w3.
        x_dvs = [xd.ap().rearrange("(io p) tb -> p io tb", p=128)
                 for xd in x_ds]
        w1H = wsb.tile([128, KI * H1], F16)
        w1L = wsb.tile([128, KI * H1], F16)
        w1h_v = w1_ds[0].ap().rearrange("(io p) h -> p io h", p=128)
        w1l_v = w1_ds[1].ap().rearrange("(io p) h -> p io h", p=128)
        xT0 = [xt_pool.tile([128, KI * ntb0], F16, tag=f"x{i}",
                            name=f"xT0_{i}")
               for i in range(3)]
        xqs = (nc.sync, nc.gpsimd, nc.scalar)
        for io in range(KI):
            for i in range(3):
                xqs[i].dma_start(out=xT0[i][:, io * ntb0:(io + 1) * ntb0],
                                 in_=x_dvs[i][:, io, 0:ntb0])
            nc.scalar.dma_start(out=w1H[:, io * H1:(io + 1) * H1],
                                in_=w1h_v[:, io, :])
        for io in range(KI):
            nc.scalar.dma_start(out=w1L[:, io * H1:(io + 1) * H1],
                                in_=w1l_v[:, io, :])
        # w2X[p, kj*H2 + h2] = w2x_dram[kj*128+p, h2]  (bf16 hi+lo)
        w2H = wsb.tile([128, J1 * H2], BF16)
        w2L = wsb.tile([128, J1 * H2], BF16)
        w2h_v = w2h_d.ap().rearrange("(kj p) h -> p kj h", p=128)
        w2l_v = w2l_d.ap().rearrange("(kj p) h -> p kj h", p=128)
        for kj in range(J1):
            nc.scalar.dma_start(out=w2H[:, kj * H2:(kj + 1) * H2],
                                in_=w2h_v[:, kj, :])
        for kj in range(J1):
            nc.scalar.dma_start(out=w2L[:, kj * H2:(kj + 1) * H2],
                                in_=w2l_v[:, kj, :])
        w3hT = wsb.tile([128, J2 * O], BF16)
        nc.scalar.dma_start(
            out=w3hT.rearrange("p (kj o) -> p kj o", kj=J2),
            in_=w3h_d.ap().rearrange("(kj p) o -> p kj o", p=128))

        # ---- scan scratch -------------------------------------------------
        # The LIF state is kept NEGATED (nmem = -mem) and written in place
        # over the cur tiles, which turns each step into two stt ops:
        #   A: tmp   = (nmem * beta) - cur          [= -(beta*mem + cur)]
        #   B: nmem' = (nmem is_lt -1) + tmp        [= -(tmp_ref - spike)]
        # fp32 RNE is sign-symmetric, so this is bit-identical to the
        # reference sequence; spikes are extracted afterwards in batched
        # is_lt ops (off the serial chain).
        tmp1 = wsb.tile([128, J1 * BL], F32)
        tmp2 = wsb.tile([128, J2 * BL], F32)
        zf1 = wsb.tile([128, J1 * BL], F32)
        zf2 = wsb.tile([128, J2 * BL], F32)
        nc.vector.memset(zf1, 0.0)
        nc.vector.memset(zf2, 0.0)
        # PE warm-up: the HAM clock gate needs ~3.4us of sustained busy to
        # reach full speed, and the PE would otherwise idle through the
        # initial x/w1 DMA fill.  Dummy matmuls on the zeroed scratch tile
        # trip the gate so chunk 0's real matmuls run at full clock.
        nwarm = opts.get("pe_warmup", 16)
        if nwarm:
            wpt = pp3.tile([64, 64], F32, tag="pp3", name="warmpt")
            for _ in range(nwarm):
                nc.tensor.matmul(wpt, lhsT=zf1[:, 0:64], rhs=zf1,
                                 start=True, stop=True)

        # Chunk c covers timesteps [toff[c], toff[c]+sched[c]); per-chunk
        # tile widths follow its size.
        toff = [0]
        for s in sched:
            toff.append(toff[-1] + s)

        # ---- per-stage emitters (software-pipelined below) ---------------
        def emit_mm1(c, first=False):
            """fp16 3-pass: cur1[h1, tb] = w1h@(xh+xl)^T + (w1l<<10)@(xh>>10)^T.
            All 12 matmuls of a j-tile accumulate in one PSUM group; the
            lo-pass operands carry opposite 2^10 exponent shifts so products
            land at natural scale (band ~2.7e-7 < mem1's fp32 half-ulp:
            measured output rel-err identical to the fp32-MM1 build).
            cur1 scan layout: col = t*64 + j*8 + b   (h1 = j*128 + p)."""
            tcsz = sched[c]
            ntb = tcsz * BL
            tb0 = toff[c] * BL
            if first:
                xs = xT0
            else:
                xs = [xt_pool.tile([128, KI * ntb], F16, tag=f"x{i}",
                                   name=f"xc{c}_{i}")
                      for i in range(3)]
                for i in range(3):
                    xqs[i].dma_start(
                        out=xs[i].rearrange("p (io tb) -> p io tb", io=KI),
                        in_=x_dvs[i][:, :, tb0:tb0 + ntb])
            cur1 = cur1_pool.tile([128, tcsz * J1 * BL], F32, tag="cur1")
            cur1_v = cur1.rearrange("p (t j b) -> p t j b", t=tcsz, j=J1, b=BL)
            for j in range(J1):
                pt = pp1.tile([128, ntb], F32, tag="pp1")
                i_mm = 0
                for io in range(KI):
                    for xi in (0, 1):  # xh, xl vs stationary w1h
                        nc.tensor.matmul(
                            pt,
                            lhsT=w1H[:, io * H1 + j * 128:
                                     io * H1 + (j + 1) * 128],
                            rhs=xs[xi][:, io * ntb:(io + 1) * ntb],
                            start=(i_mm == 0), stop=False)
                        i_mm += 1
                for io in range(KI):  # xh>>10 vs stationary w1l<<10
                    nc.tensor.matmul(
                        pt,
                        lhsT=w1L[:, io * H1 + j * 128:
                                 io * H1 + (j + 1) * 128],
                        rhs=xs[2][:, io * ntb:(io + 1) * ntb],
                        start=False, stop=(io == KI - 1))
                nc.scalar.activation(
                    cur1_v[:, :, j, :],
                    pt.rearrange("p (t b) -> p t b", b=BL), ACTF.Copy)
            return cur1

        def emit_scan1(c, cur1, prev):
            """prev = (cur1-as-nmem tile of chunk c-1, its tcsz) or None.
            Overwrites cur1 slices with the negated membrane trajectory;
            returns (spk1, cur1) so the caller can chain the next chunk."""
            tcsz = sched[c]
            W = J1 * BL
            spk1 = spk1_pool.tile([128, tcsz * W], BF16, tag="spk1")
            for t in range(tcsz):
                cs = cur1[:, t * W:(t + 1) * W]
                if t == 0:
                    nprev = zf1 if prev is None else \
                        prev[0][:, (prev[1] - 1) * W: prev[1] * W]
                else:
                    nprev = cur1[:, (t - 1) * W: t * W]
                nc.vector.scalar_tensor_tensor(
                    tmp1, nprev, BETA1, cs, ALU.mult, ALU.subtract)
                nc.vector.scalar_tensor_tensor(
                    cs, nprev, -1.0, tmp1, ALU.is_lt, ALU.add)
                if t % 16 == 15 or t == tcsz - 1:
                    g0 = (t // 16) * 16
                    nc.vector.tensor_scalar(
                        spk1[:, g0 * W:(t + 1) * W],
                        cur1[:, g0 * W:(t + 1) * W], -1.0, None, ALU.is_lt)
            return spk1, cur1

        def emit_mm2(c, spk1):
            """bf16 hi+lo: cur2[h2, tb] = w2 @ spk1^T."""
            tcsz = sched[c]
            ntb = tcsz * BL
            spk1_v = spk1.rearrange("p (t j b) -> p t j b", t=tcsz, j=J1, b=BL)
            cur2 = cur2_pool.tile([128, tcsz * J2 * BL], F32, tag="cur2")
            cur2_v = cur2.rearrange("p (t j b) -> p t j b", t=tcsz, j=J2, b=BL)
            for j in range(J2):
                pt = pp2.tile([128, ntb], F32, tag="pp2")
                i_mm = 0
                for w2X in (w2H, w2L):
                    for kj in range(J1):
                        nc.tensor.matmul(
                            pt,
                            lhsT=w2X[:, kj * H2 + j * 128: kj * H2 + (j + 1) * 128],
                            rhs=spk1_v[:, :, kj, :],
                            start=(i_mm == 0), stop=(i_mm == 2 * J1 - 1))
                        i_mm += 1
                nc.scalar.activation(
                    cur2_v[:, :, j, :],
                    pt.rearrange("p (t b) -> p t b", b=BL), ACTF.Copy)
            return cur2

        def emit_scan2(c, cur2, prev):
            """Same negated in-place scan; spk2 is written j-major
            (col = j*ntb + t*8 + b) so matmul-3's stationary operand reads
            are single-stride.  prev = (cur2-as-nmem of c-1, its tcsz)."""
            tcsz = sched[c]
            W = J2 * BL
            ntb = tcsz * BL
            spk2 = spk2_pool.tile([128, tcsz * W], BF16, tag="spk2")
            spk2_tv = spk2.rearrange("p (j t b) -> p t j b",
                                     j=J2, t=tcsz, b=BL)
            cur2_v = cur2.rearrange("p (t j b) -> p t j b",
                                    t=tcsz, j=J2, b=BL)
            for t in range(tcsz):
                cs = cur2[:, t * W:(t + 1) * W]
                if t == 0:
                    nprev = zf2 if prev is None else \
                        prev[0][:, (prev[1] - 1) * W: prev[1] * W]
                else:
                    nprev = cur2[:, (t - 1) * W: t * W]
                nc.vector.scalar_tensor_tensor(
                    tmp2, nprev, BETA2, cs, ALU.mult, ALU.subtract)
                nc.vector.scalar_tensor_tensor(
                    cs, nprev, -1.0, tmp2, ALU.is_lt, ALU.add)
                if t % 16 == 15 or t == tcsz - 1:
                    g0 = (t // 16) * 16
                    nc.vector.tensor_scalar(
                        spk2_tv[:, g0:t + 1, :, :],
                        cur2_v[:, g0:t + 1, :, :], -1.0, None, ALU.is_lt)
            return spk2, cur2

        def emit_mm3(c, spk2):
            """bf16 single pass: out[tb, o] = spk2 @ w3^T."""
            tcsz = sched[c]
            ntb = tcsz * BL
            tb0 = toff[c] * BL
            for m0 in range(0, ntb, 128):
                msz = min(128, ntb - m0)
                pt = pp3.tile([msz, O], F32, tag="pp3")
                for kj in range(J2):
                    nc.tensor.matmul(
                        pt,
                        lhsT=spk2[:, kj * ntb + m0: kj * ntb + m0 + msz],
                        rhs=w3hT[:, kj * O:(kj + 1) * O],
                        start=(kj == 0), stop=(kj == J2 - 1))
                osb = out_pool.tile([msz, O], F32, tag="osb")
                nc.scalar.activation(osb, pt, ACTF.Copy)  # b3 is zero; S+0=S
                r0 = tb0 + m0
                nc.sync.dma_start(out=y[r0:r0 + msz, :], in_=osb)

        # ---- software-pipelined emission ---------------------------------
        # DVE stream runs scan1(c+1) BEFORE scan2(c) so matmul-2(c+1) never
        # waits behind layer-2's serial chain; matmul-3(c) is deferred one
        # chunk so the PE, which executes in emission order, doesn't stall
        # on scan2(c) while matmul work is available.
        hwloop = opts.get("hwloop", 1)
        loop_cm = tc_ctx.For_i(0, hwloop, 1) if hwloop > 1 else None
        if loop_cm is not None:
            loop_cm.__enter__()
        for _r in range(opts.get("repeat", 1)):
            cur1 = emit_mm1(0, first=(_r == 0 and hwloop == 1))
            spk1, nm1 = emit_scan1(0, cur1, None)
            spk2 = {}
            nm2 = None
            for c in range(nch):
                if c + 1 < nch:
                    cur1 = emit_mm1(c + 1)
                cur2 = emit_mm2(c, spk1)
                if c + 1 < nch:
                    spk1_next, nm1_next = emit_scan1(
                        c + 1, cur1, (nm1, sched[c]))
                spk2[c], nm2 = emit_scan2(c, cur2, None if c == 0 else
                                          (nm2, sched[c - 1]))
                if c + 1 < nch:
                    spk1, nm1 = spk1_next, nm1_next
            # All matmul-3 tiles run in the epilogue: they are the only
            # PE work with no serial-scan dependency left, so they fill the
            # drain window while layer-2's last chains finish.
            for cc in range(nch):
                emit_mm3(cc, spk2.pop(cc))
        if loop_cm is not None:
            loop_cm.__exit__(None, None, None)


def prep_inputs(x, w1, w2, w3, n_t=T):
    """Host-side layout prep shared by kernel() and tests.
    Returns (per-core x dict list, common dict of weight arrays)."""
    x = np.asarray(x, dtype=np.float32)
    w1 = np.asarray(w1, dtype=np.float32)
    w2 = np.asarray(w2, dtype=np.float32)
    w3 = np.asarray(w3, dtype=np.float32)
    SC = np.float32(2.0 ** 10)
    w1t = np.ascontiguousarray(w1.T)                       # [I, H1] f32
    w1hs = w1t.astype(np.float16)
    w1ls = ((w1t - w1hs.astype(np.float32)) * SC).astype(np.float16)
    w2t = np.ascontiguousarray(w2.T)                       # [H1, H2] f32
    w2hs = w2t.astype(ml_dtypes.bfloat16)
    w2ls = (w2t - w2hs.astype(np.float32)).astype(ml_dtypes.bfloat16)
    w3t = np.ascontiguousarray(w3.T)                       # [H2, O] f32
    w3hs = w3t.astype(ml_dtypes.bfloat16)
    common = {
        "w1h": w1hs,
        "w1l10": w1ls,
        "w2h": w2hs,
        "w2l": w2ls,
        "w3h": w3hs,
    }
    xcores = []
    for cid in range(NCORES):
        xs = x[:, cid * BL:(cid + 1) * BL, :].reshape(n_t * BL, I)
        xT = np.ascontiguousarray(xs.T)                    # [I, n_t*BL] f32
        xh = xT.astype(np.float16)
        xl = (xT - xh.astype(np.float32)).astype(np.float16)
        xh10 = (xh.astype(np.float32) / SC).astype(np.float16)
        xcores.append({"xh": xh, "xl": xl, "xh10": xh10})
    return xcores, common


_NC_CACHE = {}


def _get_nc():
    if "nc" not in _NC_CACHE:
        _NC_CACHE["nc"] = build()
    return _NC_CACHE["nc"]


def kernel(x, w1, b1, w2, b2, w3, b3, **_unused):
    """Full inputs in, full output out. b1/b2/b3 are zeros in this problem
    (asserted) -- the device program skips the bias adds."""
    assert not np.any(np.asarray(b1)) and not np.any(np.asarray(b2)) \
        and not np.any(np.asarray(b3)), "nonzero biases unsupported"

    nc = _get_nc()
    xcores, common = prep_inputs(x, w1, w2, w3)
    in_maps = [{**xcores[cid], **common} for cid in range(NCORES)]
    res = run_bass_kernel_spmd(nc, in_maps, list(range(NCORES)))
    outs = [r["y"].reshape(T, BL, O) for r in res.results]
    return np.concatenate(outs, axis=1)


if __name__ == "__main__":
    nc = build()
    print("built OK")

